# revision 8
# baseline (speedup 1.0000x reference)
"""Trainium2 Bass kernel for nn_LinearKAN (histogram_binning).

Math
----
reference computes, per (batch b, out o):

    out[b,o] = sum_i  PL_interp(x[b,i]; bp[o,i,:], val[o,i,:])

where bp is the SAME sorted grid for every (o,i) (tiled linspace).  A
piecewise-linear function on a uniform grid with knots u = 0..S (where
u = (x - bp0)/h) has an exact ReLU-basis expansion:

    f(u) = val_0 + sum_{s=0..S-1} C_s * relu(u - s)
    C_0  = val_1 - val_0
    C_s  = val_{s+1} - 2*val_s + val_{s-1}     (s >= 1)

so the whole layer becomes a bias plus 20 dense matmuls with contraction
over (segment s, in-feature i):

    out[b,o] = bias[o] + sum_s sum_i C_s[o,i] * relu(u[b,i] - s)
    bias[o]  = sum_i val[o,i,0]

Device kernel (per core, SPMD over 8 cores):
  - shard: batch into 4 quarters (B_loc=256) x out-features into 2 halves
    (O_loc=128).  No cross-device reduction.
  - compute u^T tiles [i,b] on ScalarE (one affine+relu activation),
    build g_s = relu(u - s) tiles on VectorE/ScalarE,
    accumulate out^T[o,b] = sum C_s^T g_s in PSUM via 40 matmuls
    (K=128 chunks of the (s,i) contraction, fp32r at full PE rate),
    add bias during the PSUM->SBUF move, DMA out.
Host only slices/transposes/prepares C (layout prep), no heavy math.
"""

import os
import numpy as np

import concourse.bass as bass
import concourse.mybir as mybir
import concourse.tile as tile
from concourse import bacc
from concourse.bass_utils import run_bass_kernel_spmd

# Problem shape (hardcoded per the task contract).
B, O, I, S = 1024, 256, 256, 20
N_CORES = 8
B_SPLIT, O_SPLIT = 4, 2
B_LOC, O_LOC = B // B_SPLIT, O // O_SPLIT  # 256, 128
KT = 2 * S          # 40 K-tiles of 128 over the (s, i) contraction
N_CCHUNK = 5        # C DMA'd in 5 chunks of [128, 1024] (512 KB each)
F32 = mybir.dt.float32

# Matmul operand dtype: float32r streams at full PE rate (1 cyc/row for
# N>=256) vs float32's 4 cyc/row.  Switchable for accuracy fallback.
MM_DT = mybir.dt.float32r if os.environ.get("KAN_MM_DT", "f32") == "f32r" else F32


def _build_nc(scale: float, ubias: float) -> bass.Bass:
    """Build the (SPMD-identical) single-core Bass graph."""
    nc = bacc.Bacc("TRN2", target_bir_lowering=False, debug=False)

    # Register const APs for the ScalarE activation biases we use
    # (activation() lowers float biases via nc.const_aps).
    def _reg_const(v: float):
        if (F32, v) in nc.const_aps.aps:
            return
        t = nc.alloc_sbuf_tensor(f"const-f32-{v}", [128, 1], F32)
        nc.gpsimd.memset(t.ap(), v)
        nc.const_aps.aps[(F32, v)] = t.ap()

    _reg_const(float(ubias))
    for kt in range(2 * S):
        if kt % 4 == 3:
            _reg_const(-float(kt // 2))
    nc.all_engine_barrier()

    xT = nc.declare_dram_parameter("xT", [I, B_LOC], F32, isOutput=False)
    C = nc.declare_dram_parameter("C", [128, KT * 128], F32, isOutput=False)
    biasp = nc.declare_dram_parameter("biasp", [128, 1], F32, isOutput=False)
    out = nc.declare_dram_parameter("out", [O_LOC, B_LOC], F32, isOutput=True)

    cchunk_w = KT * 128 // N_CCHUNK           # 1024
    kt_per_chunk = cchunk_w // 128            # 8

    with tile.TileContext(nc) as tc:
        with (
            tc.tile_pool(name="xt", bufs=2) as xpool,
            tc.tile_pool(name="u", bufs=2) as upool,
            tc.tile_pool(name="g", bufs=KT) as gpool,
            tc.tile_pool(name="c", bufs=N_CCHUNK) as cpool,
            tc.tile_pool(name="bias", bufs=1) as bpool,
            tc.tile_pool(name="o", bufs=1) as opool,
            tc.tile_pool(name="ps", bufs=1, space="PSUM") as pspool,
        ):
            # --- DMA in ---
            xt = []
            for ih in range(2):
                t = xpool.tile([128, B_LOC], F32, tag="xt")
                nc.sync.dma_start(t[:], xT[ih * 128:(ih + 1) * 128, :])
                xt.append(t)
            bias_sb = bpool.tile([128, 1], F32)
            nc.sync.dma_start(bias_sb[:], biasp[:])
            cchunks = []
            for cc in range(N_CCHUNK):
                t = cpool.tile([128, cchunk_w], F32, tag="c")
                nc.sync.dma_start(t[:], C[:, cc * cchunk_w:(cc + 1) * cchunk_w])
                cchunks.append(t)

            # --- u = relu(scale*x + ubias)  (ScalarE, one op per i-half) ---
            u = []
            for ih in range(2):
                t = upool.tile([128, B_LOC], F32, tag="u")
                nc.scalar.activation(
                    t[:], xt[ih][:], mybir.ActivationFunctionType.Relu,
                    bias=float(ubias), scale=float(scale),
                )
                u.append(t)

            # --- g_s = relu(u - s) + accumulate matmuls ---
            ps = pspool.tile([O_LOC, B_LOC], F32)
            for kt in range(KT):
                s, ih = kt // 2, kt % 2
                if s == 0:
                    rhs = u[ih]  # relu(u - 0) == u  (u >= 0)
                else:
                    rhs = gpool.tile([128, B_LOC], F32, tag="g")
                    if kt % 4 == 3:  # ~1/4 of builds on ScalarE, rest VectorE
                        nc.scalar.activation(
                            rhs[:], u[ih][:], mybir.ActivationFunctionType.Relu,
                            bias=-float(s), scale=1.0,
                        )
                    else:
                        nc.vector.tensor_scalar(
                            rhs[:], u[ih][:], float(s), 0.0,
                            mybir.AluOpType.subtract, mybir.AluOpType.max,
                        )
                lhsT = cchunks[kt // kt_per_chunk][
                    :, (kt % kt_per_chunk) * 128:(kt % kt_per_chunk + 1) * 128
                ]
                nc.tensor.matmul(
                    ps[:], lhsT.bitcast(MM_DT), rhs[:].bitcast(MM_DT),
                    start=(kt == 0), stop=(kt == KT - 1),
                )

            # --- out = ps + bias (per-partition scalar), then DMA out ---
            out_sb = opool.tile([O_LOC, B_LOC], F32)
            nc.vector.tensor_scalar(
                out_sb[:], ps[:], bias_sb[:, 0:1], None, mybir.AluOpType.add,
            )
            nc.sync.dma_start(out[:], out_sb[:])
    nc.compile()
    return nc


_NC_CACHE: dict = {}


def _get_nc(scale: float, ubias: float) -> bass.Bass:
    key = (float(scale), float(ubias), str(MM_DT))
    if key not in _NC_CACHE:
        _NC_CACHE[key] = _build_nc(scale, ubias)
    return _NC_CACHE[key]


def prepare(x: np.ndarray, breakpoints: np.ndarray, values: np.ndarray):
    """Host prep: build the Bass graph (cached) + per-core input maps."""
    x = np.asarray(x, np.float32)
    breakpoints = np.asarray(breakpoints, np.float32)
    values = np.asarray(values, np.float32)

    # Grid affine params from the (shared) breakpoint row.
    bpr = breakpoints[0, 0].astype(np.float64)
    h = (bpr[-1] - bpr[0]) / S
    scale = float(1.0 / h)
    ubias = float(-bpr[0] / h)

    # ReLU-basis coefficients (host = layout prep + finite differences).
    Vf = values  # [O, I, S+1]
    Cs = np.empty((S, O, I), np.float32)
    Cs[0] = Vf[:, :, 1] - Vf[:, :, 0]
    for s in range(1, S):
        Cs[s] = Vf[:, :, s + 1] - 2.0 * Vf[:, :, s] + Vf[:, :, s - 1]
    bias_o = Vf[:, :, 0].sum(axis=1, dtype=np.float64).astype(np.float32)  # [O]

    # Per-core C layout [j, kt, o]: kt = 2*s + ih, j = i within half,
    # o = out-feature within this core's half.
    # Cr[s, oh, o, ih, j] view of Cs[s, O, I]:
    Cr = Cs.reshape(S, O_SPLIT, O_LOC, 2, 128)
    xT_full = np.ascontiguousarray(x.T)  # [I, B]

    in_maps = []
    for c in range(N_CORES):
        bq, oh = c % B_SPLIT, c // B_SPLIT
        xT_c = np.ascontiguousarray(xT_full[:, bq * B_LOC:(bq + 1) * B_LOC])
        # [s, o, ih, j] -> [j, s, ih, o] -> [128, KT*128]
        C_c = np.ascontiguousarray(
            Cr[:, oh].transpose(3, 0, 2, 1)
        ).reshape(128, KT * 128)
        bias_c = np.ascontiguousarray(
            bias_o[oh * O_LOC:(oh + 1) * O_LOC].reshape(128, 1)
        )
        in_maps.append({"xT": xT_c, "C": C_c, "biasp": bias_c})

    nc = _get_nc(scale, ubias)
    return nc, in_maps


def kernel(x: np.ndarray, breakpoints: np.ndarray, values: np.ndarray,
           **_extra) -> np.ndarray:
    nc, in_maps = prepare(x, breakpoints, values)
    res = run_bass_kernel_spmd(nc, in_maps, list(range(N_CORES)))

    outf = np.empty((B, O), np.float32)
    for c in range(N_CORES):
        bq, oh = c % B_SPLIT, c // B_SPLIT
        outf[bq * B_LOC:(bq + 1) * B_LOC, oh * O_LOC:(oh + 1) * O_LOC] = \
            res.results[c]["out"].T
    return outf


if __name__ == "__main__":
    rng = np.random.default_rng(0)
    x = rng.uniform(-1, 1, (B, I)).astype(np.float32)
    bp = np.tile(np.linspace(-1, 1, S + 1, dtype=np.float32), (O, I, 1))
    v = (rng.standard_normal((O, I, S + 1)) * 0.1).astype(np.float32)
    out = kernel(x, bp, v)
    print("kernel ran, out:", out.shape, out.dtype, float(out.std()))


# revision 10
# speedup vs baseline: 1.1725x; 1.1725x over previous
"""Trainium2 Bass kernel for nn_LinearKAN (histogram_binning).

Math
----
reference computes, per (batch b, out o):

    out[b,o] = sum_i  PL_interp(x[b,i]; bp[o,i,:], val[o,i,:])

where bp is the SAME sorted uniform grid for every (o,i) (tiled
linspace).  With u = (x - bp0)/h in [0, S), the piecewise-linear
interpolant has an exact *clamp basis* expansion

    f(u) = val_0 + sum_{s=0..S-1} M_s * clamp(u - s, 0, 1)
    M_s  = val_{s+1} - val_s              (segment slopes)

so the layer is a bias plus S dense matmuls contracting over (s, i):

    out[b,o] = bias[o] + sum_s sum_i M_s[o,i] * r_s[b,i]
    r_s      = clamp(u - s, 0, 1),   bias[o] = sum_i val[o,i,0]

The clamp basis quantizes benignly: r entries are exactly 0, exactly 1,
or the single fractional t per (b,i) -- so fp16 operands lose almost
nothing.  The slopes are split M = M_hi + M_lo/2048 with both parts
fp16 (2048 scaling keeps M_lo out of fp16-denormal range), accumulated
into two PSUM groups and combined in the tail:
out = ps_hi + 2^-11 * ps_lo + bias.  Measured ~2e-4 rel err.

Device kernel (per core, SPMD over 8 cores):
  - shard batch into 4 quarters (B_loc=256) x out-features into 2
    halves (O_loc=128); no cross-device reduction.
  - u^T [i, (ih,b)] via one ScalarE activation; r_s tiles [128, 512]
    fp16 via VectorE/ScalarE (relu then min-1); 2x40 fp16 matmuls
    (K=128 chunks of the (s,i) contraction) at full PE rate; tail
    DVE combine + bias; DMA out.
Host only slices/transposes/differences the params (layout prep).
"""

import os
import numpy as np

import concourse.bass as bass
import concourse.mybir as mybir
import concourse.tile as tile
from concourse import bacc
from concourse.bass_utils import run_bass_kernel_spmd

# Problem shape (hardcoded per the task contract).
B, O, I, S = 1024, 256, 256, 20
N_CORES = 8
B_SPLIT, O_SPLIT = 4, 2
B_LOC, O_LOC = B // B_SPLIT, O // O_SPLIT  # 256, 128
KT = 2 * S          # 40 K-tiles of 128 over the (s, i) contraction
N_CCHUNK = 2        # each C array DMA'd in 2 chunks
LO_SCALE = 2048.0   # M_lo pre-scale (keeps fp16 normal); undone in tail
F32 = mybir.dt.float32
F16 = mybir.dt.float16
FW = 2 * B_LOC      # r/u tile free width: both i-halves side by side

# s values whose relu step runs on ScalarE (rest on VectorE); the min-1
# step always runs on VectorE.  Balances ~9.5us on each engine.
ACT_RELU_S = set(range(6, 20))


def _build_nc(scale: float, ubias: float) -> bass.Bass:
    """Build the (SPMD-identical) single-core Bass graph."""
    nc = bacc.Bacc("TRN2", target_bir_lowering=False, debug=False)

    xT = nc.declare_dram_parameter("xT", [128, FW], F32, isOutput=False)
    Chi = nc.declare_dram_parameter("Chi", [128, KT * 128], F16, isOutput=False)
    Clo = nc.declare_dram_parameter("Clo", [128, KT * 128], F16, isOutput=False)
    # btab: col0 = output bias (per o-partition), col1 = ubias, col 2+k = -(k+1)
    btab = nc.declare_dram_parameter("btab", [128, 24], F32, isOutput=False)
    out = nc.declare_dram_parameter("out", [O_LOC, B_LOC], F32, isOutput=True)

    cw = KT * 128 // N_CCHUNK
    kt_pc = cw // 128

    with tile.TileContext(nc) as tc:
        with (
            tc.tile_pool(name="xt", bufs=1) as xpool,
            tc.tile_pool(name="u", bufs=1) as upool,
            tc.tile_pool(name="w", bufs=4) as wpool,
            tc.tile_pool(name="r", bufs=S) as rpool,
            tc.tile_pool(name="c", bufs=2 * N_CCHUNK) as cpool,
            tc.tile_pool(name="b", bufs=1) as bpool,
            tc.tile_pool(name="o", bufs=2) as opool,
            tc.tile_pool(name="ps", bufs=2, space="PSUM") as pspool,
        ):
            # --- DMA in (sync queue: xT, btab, Chi; gpsimd queue: Clo) ---
            btab_sb = bpool.tile([128, 24], F32)
            nc.sync.dma_start(btab_sb[:], btab[:])
            xt = xpool.tile([128, FW], F32)
            nc.sync.dma_start(xt[:], xT[:])
            chi = []
            for cc in range(N_CCHUNK):
                t = cpool.tile([128, cw], F16, tag="chi")
                nc.sync.dma_start(t[:], Chi[:, cc * cw:(cc + 1) * cw])
                chi.append(t)
            clo = []
            for cc in range(N_CCHUNK):
                t = cpool.tile([128, cw], F16, tag="clo")
                nc.gpsimd.dma_start(t[:], Clo[:, cc * cw:(cc + 1) * cw])
                clo.append(t)

            # ACT table preload: cheap Copy on the (early, tiny) btab tile.
            dummy = wpool.tile([128, 1], F32, tag="dummy")
            nc.scalar.copy(dummy[:], btab_sb[:, 0:1])

            # --- u = relu(scale*x + ubias), one op over both i-halves ---
            u2 = upool.tile([128, FW], F32)
            nc.scalar.activation(
                u2[:], xt[:], mybir.ActivationFunctionType.Relu,
                bias=btab_sb[:, 1:2], scale=float(scale),
            )

            # --- r_s = clamp(u - s, 0, 1) in fp16 ---
            r = []
            for s in range(S):
                rs = rpool.tile([128, FW], F16, tag="r")
                if s == 0:
                    # u >= 0, so clamp(u,0,1) = min(u,1)
                    nc.vector.tensor_scalar(
                        rs[:], u2[:], 1.0, None, mybir.AluOpType.min)
                elif s == S - 1:
                    # u < 20, so clamp(u-19,0,1) = relu(u-19)
                    nc.scalar.activation(
                        rs[:], u2[:], mybir.ActivationFunctionType.Relu,
                        bias=btab_sb[:, 1 + s:2 + s], scale=1.0)
                elif s in ACT_RELU_S:
                    w = wpool.tile([128, FW], F32, tag="w")
                    nc.scalar.activation(
                        w[:], u2[:], mybir.ActivationFunctionType.Relu,
                        bias=btab_sb[:, 1 + s:2 + s], scale=1.0)
                    nc.vector.tensor_scalar(
                        rs[:], w[:], 1.0, None, mybir.AluOpType.min)
                else:
                    w = wpool.tile([128, FW], F32, tag="w")
                    nc.vector.tensor_scalar(
                        w[:], u2[:], float(s), float(s),
                        mybir.AluOpType.max, mybir.AluOpType.subtract)
                    nc.vector.tensor_scalar(
                        rs[:], w[:], 1.0, None, mybir.AluOpType.min)
                r.append(rs)

            # --- matmuls: hi sweep then lo sweep, two PSUM groups ---
            ps_hi = pspool.tile([O_LOC, B_LOC], F32, tag="ph")
            ps_lo = pspool.tile([O_LOC, B_LOC], F32, tag="pl")
            for grp, (ps, carr) in enumerate(((ps_hi, chi), (ps_lo, clo))):
                for kt in range(KT):
                    s, ih = kt // 2, kt % 2
                    lhsT = carr[kt // kt_pc][
                        :, (kt % kt_pc) * 128:(kt % kt_pc + 1) * 128]
                    rhs = r[s][:, ih * B_LOC:(ih + 1) * B_LOC]
                    nc.tensor.matmul(ps[:], lhsT, rhs,
                                     start=(kt == 0), stop=(kt == KT - 1))

            # --- tail: out = ps_hi + ps_lo/2048 + bias ---
            t1 = opool.tile([O_LOC, B_LOC], F32, tag="t1")
            nc.vector.tensor_scalar(
                t1[:], ps_lo[:], 1.0 / LO_SCALE, btab_sb[:, 0:1],
                mybir.AluOpType.mult, mybir.AluOpType.add)
            out_sb = opool.tile([O_LOC, B_LOC], F32, tag="osb")
            nc.vector.tensor_tensor(
                out_sb[:], ps_hi[:], t1[:], mybir.AluOpType.add)
            nc.sync.dma_start(out[:], out_sb[:])
    nc.compile()
    return nc


_NC_CACHE: dict = {}


def _get_nc(scale: float, ubias: float) -> bass.Bass:
    key = (float(scale), float(ubias))
    if key not in _NC_CACHE:
        _NC_CACHE[key] = _build_nc(scale, ubias)
    return _NC_CACHE[key]


def prepare(x: np.ndarray, breakpoints: np.ndarray, values: np.ndarray):
    """Host prep: build the Bass graph (cached) + per-core input maps."""
    x = np.asarray(x, np.float32)
    breakpoints = np.asarray(breakpoints, np.float32)
    values = np.asarray(values, np.float32)

    # Grid affine params from the (shared) breakpoint row.
    bpr = breakpoints[0, 0].astype(np.float64)
    h = (bpr[-1] - bpr[0]) / S
    scale = float(1.0 / h)
    ubias = float(-bpr[0] / h)

    # Clamp-basis slopes, split into fp16 hi + scaled fp16 lo.
    Vf = values  # [O, I, S+1]
    M = (Vf[:, :, 1:] - Vf[:, :, :-1]).transpose(2, 0, 1)  # [S, O, I] f32
    M = np.ascontiguousarray(M, np.float32)
    Mhi = M.astype(np.float16)
    Mlo = ((M - Mhi.astype(np.float32)) * LO_SCALE).astype(np.float16)
    bias_o = Vf[:, :, 0].sum(axis=1, dtype=np.float64).astype(np.float32)

    # Per-core layouts.
    #   C*: [j, kt, o] fp16 with kt = 2*s + ih, j = i within half.
    #   xT: [j, ih*B_LOC + b] fp32.
    Mhi_r = Mhi.reshape(S, O_SPLIT, O_LOC, 2, 128)  # [s, oh, o, ih, j]
    Mlo_r = Mlo.reshape(S, O_SPLIT, O_LOC, 2, 128)
    xr = x.reshape(B_SPLIT, B_LOC, 2, 128)          # [bq, b, ih, j]

    in_maps = []
    for c in range(N_CORES):
        bq, oh = c % B_SPLIT, c // B_SPLIT
        # xr[bq] axes (b, ih, j) -> (j, ih, b) -> [128, FW]
        xT_c = np.ascontiguousarray(
            xr[bq].transpose(2, 1, 0)).reshape(128, FW)
        C_hi = np.ascontiguousarray(
            Mhi_r[:, oh].transpose(3, 0, 2, 1)).reshape(128, KT * 128)
        C_lo = np.ascontiguousarray(
            Mlo_r[:, oh].transpose(3, 0, 2, 1)).reshape(128, KT * 128)
        bt = np.empty((128, 24), np.float32)
        bt[:, 0] = bias_o[oh * O_LOC:(oh + 1) * O_LOC]
        bt[:, 1] = ubias
        for k in range(1, 23):
            bt[:, 1 + k] = -float(k)
        in_maps.append({"xT": xT_c, "Chi": C_hi, "Clo": C_lo, "btab": bt})

    nc = _get_nc(scale, ubias)
    return nc, in_maps


def kernel(x: np.ndarray, breakpoints: np.ndarray, values: np.ndarray,
           **_extra) -> np.ndarray:
    nc, in_maps = prepare(x, breakpoints, values)
    res = run_bass_kernel_spmd(nc, in_maps, list(range(N_CORES)))

    outf = np.empty((B, O), np.float32)
    for c in range(N_CORES):
        bq, oh = c % B_SPLIT, c // B_SPLIT
        outf[bq * B_LOC:(bq + 1) * B_LOC, oh * O_LOC:(oh + 1) * O_LOC] = \
            res.results[c]["out"].T
    return outf


if __name__ == "__main__":
    rng = np.random.default_rng(0)
    x = rng.uniform(-1, 1, (B, I)).astype(np.float32)
    bp = np.tile(np.linspace(-1, 1, S + 1, dtype=np.float32), (O, I, 1))
    v = (rng.standard_normal((O, I, S + 1)) * 0.1).astype(np.float32)
    out = kernel(x, bp, v)
    print("kernel ran, out:", out.shape, out.dtype, float(out.std()))


# revision 14
# speedup vs baseline: 1.2637x; 1.0777x over previous
"""Trainium2 Bass kernel for nn_LinearKAN (histogram_binning).

Math
----
reference computes, per (batch b, out o):

    out[b,o] = sum_i  PL_interp(x[b,i]; bp[o,i,:], val[o,i,:])

where bp is the SAME sorted uniform grid for every (o,i) (tiled
linspace).  With u = (x - bp0)/h in [0, S), the piecewise-linear
interpolant has an exact *clamp basis* expansion

    f(u) = val_0 + sum_{s=0..S-1} M_s * clamp(u - s, 0, 1)
    M_s  = val_{s+1} - val_s              (segment slopes)

so the layer is a bias plus S dense matmuls contracting over (s, i):

    out[b,o] = bias[o] + sum_s sum_i M_s[o,i] * r_s[b,i]
    r_s      = clamp(u - s, 0, 1),   bias[o] = sum_i val[o,i,0]

The clamp basis quantizes benignly: r entries are exactly 0, exactly 1,
or the single fractional t per (b,i) -- so fp16 operands lose almost
nothing.  The slopes are split M = M_hi + M_lo/2048 with both parts
fp16 (2048 scaling keeps M_lo out of fp16-denormal range), accumulated
into two PSUM groups and combined in the tail:
out = ps_hi + 2^-11 * ps_lo + bias.  Measured ~2e-4 rel err.

Device kernel (per core, SPMD over 8 cores):
  - shard batch into 4 quarters (B_loc=256) x out-features into 2
    halves (O_loc=128); no cross-device reduction.
  - u^T [i, (ih,b)] via one ScalarE activation; r_s tiles [128, 512]
    fp16 via VectorE/ScalarE (relu then min-1); 2x40 fp16 matmuls
    (K=128 chunks of the (s,i) contraction) at full PE rate; tail
    DVE combine + bias; DMA out.
Host only slices/transposes/differences the params (layout prep).
"""

import os
import numpy as np

import concourse.bass as bass
import concourse.mybir as mybir
import concourse.tile as tile
from concourse import bacc
from concourse.bass_utils import run_bass_kernel_spmd

# Problem shape (hardcoded per the task contract).
B, O, I, S = 1024, 256, 256, 20
N_CORES = 8
B_SPLIT, O_SPLIT = 4, 2
B_LOC, O_LOC = B // B_SPLIT, O // O_SPLIT  # 256, 128
KT = 2 * S          # 40 K-tiles of 128 over the (s, i) contraction
CHUNK_KT = (8, 16, 16)  # C DMA chunk sizes in kt units (small first chunk)
LO_SCALE = 2048.0   # M_lo pre-scale (keeps fp16 normal); undone in tail
F32 = mybir.dt.float32
F16 = mybir.dt.float16
FW = 2 * B_LOC      # r/u tile free width: both i-halves side by side

# s values whose relu step runs on ScalarE (rest on VectorE); the min-1
# step always runs on VectorE.
ACT_RELU_S = set(range(9, 20))


def _build_nc(scale: float, ubias: float) -> bass.Bass:
    """Build the (SPMD-identical) single-core Bass graph."""
    nc = bacc.Bacc("TRN2", target_bir_lowering=False, debug=False)

    xT = nc.declare_dram_parameter("xT", [128, FW], F32, isOutput=False)
    Chi = nc.declare_dram_parameter("Chi", [128, KT * 128], F16, isOutput=False)
    Clo = nc.declare_dram_parameter("Clo", [128, KT * 128], F16, isOutput=False)
    # btab: col0 = output bias (per o-partition), col1 = ubias, col 2+k = -(k+1)
    btab = nc.declare_dram_parameter("btab", [128, 24], F32, isOutput=False)
    out = nc.declare_dram_parameter("out", [O_LOC, B_LOC], F32, isOutput=True)

    with tile.TileContext(nc) as tc:
        with (
            tc.tile_pool(name="xt", bufs=1) as xpool,
            tc.tile_pool(name="u", bufs=1) as upool,
            tc.tile_pool(name="w", bufs=4) as wpool,
            tc.tile_pool(name="r", bufs=S) as rpool,
            tc.tile_pool(name="c", bufs=2 * len(CHUNK_KT)) as cpool,
            tc.tile_pool(name="b", bufs=1) as bpool,
            tc.tile_pool(name="o", bufs=2) as opool,
            tc.tile_pool(name="ps", bufs=2, space="PSUM") as pspool,
        ):
            # --- DMA in, all on the sync HWDGE queue; order matters:
            # btab (tiny, unlocks ACT table preload), first small Chi/Clo
            # chunks (unlock matmuls), xT (unlocks u), rest of C.
            btab_sb = bpool.tile([128, 24], F32)
            nc.sync.dma_start(btab_sb[:], btab[:])
            xt = xpool.tile([128, FW], F32)
            chi = {}
            clo = {}

            def c_chunks():
                kt0 = 0
                for ci, nkt in enumerate(CHUNK_KT):
                    yield ci, kt0, nkt
                    kt0 += nkt

            for ci, kt0, nkt in c_chunks():
                th = cpool.tile([128, nkt * 128], F16, tag=f"chi{ci}")
                nc.sync.dma_start(th[:], Chi[:, kt0 * 128:(kt0 + nkt) * 128])
                if ci == 0:
                    nc.sync.dma_start(xt[:], xT[:])
                tl = cpool.tile([128, nkt * 128], F16, tag=f"clo{ci}")
                nc.sync.dma_start(tl[:], Clo[:, kt0 * 128:(kt0 + nkt) * 128])
                for k in range(nkt):
                    chi[kt0 + k] = th[:, k * 128:(k + 1) * 128]
                    clo[kt0 + k] = tl[:, k * 128:(k + 1) * 128]

            # ACT table preload: cheap Copy on the (early, tiny) btab tile.
            dummy = wpool.tile([128, 1], F32, tag="dummy")
            nc.scalar.copy(dummy[:], btab_sb[:, 0:1])

            # --- u = relu(scale*x + ubias), one op over both i-halves ---
            u2 = upool.tile([128, FW], F32)
            nc.scalar.activation(
                u2[:], xt[:], mybir.ActivationFunctionType.Relu,
                bias=btab_sb[:, 1:2], scale=float(scale),
            )

            # --- r_s = clamp(u - s, 0, 1) in fp16 ---
            r = []
            for s in range(S):
                rs = rpool.tile([128, FW], F16, tag="r")
                if s == 0:
                    # u >= 0, so clamp(u,0,1) = min(u,1)
                    nc.vector.tensor_scalar(
                        rs[:], u2[:], 1.0, None, mybir.AluOpType.min)
                elif s == S - 1:
                    # u < 20, so clamp(u-19,0,1) = relu(u-19)
                    nc.scalar.activation(
                        rs[:], u2[:], mybir.ActivationFunctionType.Relu,
                        bias=btab_sb[:, 1 + s:2 + s], scale=1.0)
                elif s in ACT_RELU_S:
                    # fp16 intermediate: values >= 1 still clamp to exactly
                    # 1.0 after quantization, t-entries keep fp16 precision,
                    # and the 16-bit input lets the min run in DVE 4x mode.
                    w = wpool.tile([128, FW], F16, tag="w")
                    nc.scalar.activation(
                        w[:], u2[:], mybir.ActivationFunctionType.Relu,
                        bias=btab_sb[:, 1 + s:2 + s], scale=1.0)
                    nc.vector.tensor_scalar(
                        rs[:], w[:], 1.0, None, mybir.AluOpType.min)
                else:
                    w = wpool.tile([128, FW], F16, tag="w")
                    nc.vector.tensor_scalar(
                        w[:], u2[:], float(s), float(s),
                        mybir.AluOpType.max, mybir.AluOpType.subtract)
                    nc.vector.tensor_scalar(
                        rs[:], w[:], 1.0, None, mybir.AluOpType.min)
                r.append(rs)

            # --- matmuls: hi/lo interleaved per kt, two PSUM groups ---
            ps_hi = pspool.tile([O_LOC, B_LOC], F32, tag="ph")
            ps_lo = pspool.tile([O_LOC, B_LOC], F32, tag="pl")
            if os.environ.get("KAN_INTERLEAVE", "1") == "1":
                for kt in range(KT):
                    s, ih = kt // 2, kt % 2
                    rhs = r[s][:, ih * B_LOC:(ih + 1) * B_LOC]
                    nc.tensor.matmul(ps_hi[:], chi[kt], rhs,
                                     start=(kt == 0), stop=(kt == KT - 1))
                    nc.tensor.matmul(ps_lo[:], clo[kt], rhs,
                                     start=(kt == 0), stop=(kt == KT - 1))
            else:
                for ps, carr in ((ps_hi, chi), (ps_lo, clo)):
                    for kt in range(KT):
                        s, ih = kt // 2, kt % 2
                        rhs = r[s][:, ih * B_LOC:(ih + 1) * B_LOC]
                        nc.tensor.matmul(ps[:], carr[kt], rhs,
                                         start=(kt == 0), stop=(kt == KT - 1))

            # --- tail: out = ps_hi + ps_lo/2048 + bias ---
            t1 = opool.tile([O_LOC, B_LOC], F32, tag="t1")
            nc.vector.tensor_scalar(
                t1[:], ps_lo[:], 1.0 / LO_SCALE, btab_sb[:, 0:1],
                mybir.AluOpType.mult, mybir.AluOpType.add)
            out_sb = opool.tile([O_LOC, B_LOC], F32, tag="osb")
            nc.vector.tensor_tensor(
                out_sb[:], ps_hi[:], t1[:], mybir.AluOpType.add)
            nc.sync.dma_start(out[:], out_sb[:])
    nc.compile()
    return nc


_NC_CACHE: dict = {}


def _get_nc(scale: float, ubias: float) -> bass.Bass:
    key = (float(scale), float(ubias))
    if key not in _NC_CACHE:
        _NC_CACHE[key] = _build_nc(scale, ubias)
    return _NC_CACHE[key]


def prepare(x: np.ndarray, breakpoints: np.ndarray, values: np.ndarray):
    """Host prep: build the Bass graph (cached) + per-core input maps."""
    x = np.asarray(x, np.float32)
    breakpoints = np.asarray(breakpoints, np.float32)
    values = np.asarray(values, np.float32)

    # Grid affine params from the (shared) breakpoint row.
    bpr = breakpoints[0, 0].astype(np.float64)
    h = (bpr[-1] - bpr[0]) / S
    scale = float(1.0 / h)
    ubias = float(-bpr[0] / h)

    # Clamp-basis slopes, split into fp16 hi + scaled fp16 lo.
    Vf = values  # [O, I, S+1]
    M = (Vf[:, :, 1:] - Vf[:, :, :-1]).transpose(2, 0, 1)  # [S, O, I] f32
    M = np.ascontiguousarray(M, np.float32)
    Mhi = M.astype(np.float16)
    Mlo = ((M - Mhi.astype(np.float32)) * LO_SCALE).astype(np.float16)
    bias_o = Vf[:, :, 0].sum(axis=1, dtype=np.float64).astype(np.float32)

    # Per-core layouts.
    #   C*: [j, kt, o] fp16 with kt = 2*s + ih, j = i within half.
    #   xT: [j, ih*B_LOC + b] fp32.
    Mhi_r = Mhi.reshape(S, O_SPLIT, O_LOC, 2, 128)  # [s, oh, o, ih, j]
    Mlo_r = Mlo.reshape(S, O_SPLIT, O_LOC, 2, 128)
    xr = x.reshape(B_SPLIT, B_LOC, 2, 128)          # [bq, b, ih, j]

    in_maps = []
    for c in range(N_CORES):
        bq, oh = c % B_SPLIT, c // B_SPLIT
        # xr[bq] axes (b, ih, j) -> (j, ih, b) -> [128, FW]
        xT_c = np.ascontiguousarray(
            xr[bq].transpose(2, 1, 0)).reshape(128, FW)
        C_hi = np.ascontiguousarray(
            Mhi_r[:, oh].transpose(3, 0, 2, 1)).reshape(128, KT * 128)
        C_lo = np.ascontiguousarray(
            Mlo_r[:, oh].transpose(3, 0, 2, 1)).reshape(128, KT * 128)
        bt = np.empty((128, 24), np.float32)
        bt[:, 0] = bias_o[oh * O_LOC:(oh + 1) * O_LOC]
        bt[:, 1] = ubias
        for k in range(1, 23):
            bt[:, 1 + k] = -float(k)
        in_maps.append({"xT": xT_c, "Chi": C_hi, "Clo": C_lo, "btab": bt})

    nc = _get_nc(scale, ubias)
    return nc, in_maps


def kernel(x: np.ndarray, breakpoints: np.ndarray, values: np.ndarray,
           **_extra) -> np.ndarray:
    nc, in_maps = prepare(x, breakpoints, values)
    res = run_bass_kernel_spmd(nc, in_maps, list(range(N_CORES)))

    outf = np.empty((B, O), np.float32)
    for c in range(N_CORES):
        bq, oh = c % B_SPLIT, c // B_SPLIT
        outf[bq * B_LOC:(bq + 1) * B_LOC, oh * O_LOC:(oh + 1) * O_LOC] = \
            res.results[c]["out"].T
    return outf


if __name__ == "__main__":
    rng = np.random.default_rng(0)
    x = rng.uniform(-1, 1, (B, I)).astype(np.float32)
    bp = np.tile(np.linspace(-1, 1, S + 1, dtype=np.float32), (O, I, 1))
    v = (rng.standard_normal((O, I, S + 1)) * 0.1).astype(np.float32)
    out = kernel(x, bp, v)
    print("kernel ran, out:", out.shape, out.dtype, float(out.std()))


# revision 20
# speedup vs baseline: 1.2661x; 1.0019x over previous
"""Trainium2 Bass kernel for nn_LinearKAN (histogram_binning).

Math
----
reference computes, per (batch b, out o):

    out[b,o] = sum_i  PL_interp(x[b,i]; bp[o,i,:], val[o,i,:])

where bp is the SAME sorted uniform grid for every (o,i) (tiled
linspace).  With u = (x - bp0)/h in [0, S), the piecewise-linear
interpolant has an exact *clamp basis* expansion

    f(u) = val_0 + sum_{s=0..S-1} M_s * clamp(u - s, 0, 1)
    M_s  = val_{s+1} - val_s              (segment slopes)

so the layer is a bias plus S dense matmuls contracting over (s, i):

    out[b,o] = bias[o] + sum_s sum_i M_s[o,i] * r_s[b,i]
    r_s      = clamp(u - s, 0, 1),   bias[o] = sum_i val[o,i,0]

The clamp basis quantizes benignly: r entries are exactly 0, exactly 1,
or the single fractional t per (b,i) -- so fp16 operands lose almost
nothing.  The slopes are split M = M_hi + M_lo/2048 with both parts
fp16 (2048 scaling keeps M_lo out of fp16-denormal range), accumulated
into two PSUM groups and combined in the tail:
out = ps_hi + 2^-11 * ps_lo + bias.  Measured ~2e-4 rel err.

Device kernel (per core, SPMD over 8 cores):
  - shard batch into 4 quarters (B_loc=256) x out-features into 2
    halves (O_loc=128); no cross-device reduction.
  - u^T [i, (ih,b)] via one ScalarE activation; r_s tiles [128, 512]
    fp16 via VectorE/ScalarE (relu then min-1); 2x40 fp16 matmuls
    (K=128 chunks of the (s,i) contraction) at full PE rate; tail
    DVE combine + bias; DMA out.
Host only slices/transposes/differences the params (layout prep).
"""

import os
import numpy as np

import concourse.bass as bass
import concourse.mybir as mybir
import concourse.tile as tile
from concourse import bacc
from concourse.bass_utils import run_bass_kernel_spmd

# Problem shape (hardcoded per the task contract).
B, O, I, S = 1024, 256, 256, 20
N_CORES = 8
B_SPLIT, O_SPLIT = 4, 2
B_LOC, O_LOC = B // B_SPLIT, O // O_SPLIT  # 256, 128
KT = 2 * S          # 40 K-tiles of 128 over the (s, i) contraction
CHUNK_KT = (8, 16, 16)  # C DMA chunk sizes in kt units (small first chunk)
LO_SCALE = 2048.0   # M_lo pre-scale (keeps fp16 normal); undone in tail
F32 = mybir.dt.float32
F16 = mybir.dt.float16
FW = 2 * B_LOC      # r/u tile free width: both i-halves side by side

# s values whose relu step runs on ScalarE (rest on VectorE); the min-1
# step always runs on VectorE.
ACT_RELU_S = set(range(8, 20))
N_WARMUP_MM = int(os.environ.get("KAN_WARMUP", "6"))  # PE HAM warmup dummies
N_GPS = int(os.environ.get("KAN_GPS", "0"))  # s-values built on GpSimd


def _strip_init_boilerplate(nc) -> None:
    """Drop the Bass-init const-AP memsets + all-engine barrier (~1.5us of
    preamble).  This kernel never reads the const APs (all activation biases
    are explicit APs), so the memsets and their barrier are dead weight."""
    blk = nc.m.functions[0].blocks[0]
    drop = (mybir.InstMemset, mybir.InstDrain, mybir.InstEventSemaphore)
    keep = [i for i in blk.instructions if not isinstance(i, drop)]
    del blk.instructions[:]
    for i in keep:
        blk.instructions.append(i)
    nc.const_aps.aps.clear()


def _build_nc(scale: float, ubias: float) -> bass.Bass:
    """Build the (SPMD-identical) single-core Bass graph."""
    nc = bacc.Bacc("TRN2", target_bir_lowering=False, debug=False)
    _strip_init_boilerplate(nc)

    xT = nc.declare_dram_parameter("xT", [128, FW], F32, isOutput=False)
    C2 = nc.declare_dram_parameter("C2", [128, 2 * KT * 128], F16,
                                   isOutput=False)
    # btab: col0 = output bias (per o-partition), col1 = ubias, col 2+k = -(k+1)
    btab = nc.declare_dram_parameter("btab", [128, 24], F32, isOutput=False)
    out = nc.declare_dram_parameter("out", [O_LOC, B_LOC], F32, isOutput=True)

    with tile.TileContext(nc) as tc:
        with (
            tc.tile_pool(name="xt", bufs=1) as xpool,
            tc.tile_pool(name="u", bufs=1) as upool,
            tc.tile_pool(name="w", bufs=4) as wpool,
            tc.tile_pool(name="r", bufs=S) as rpool,
            tc.tile_pool(name="c", bufs=1) as cpool,
            tc.tile_pool(name="b", bufs=1) as bpool,
            tc.tile_pool(name="o", bufs=2) as opool,
            tc.tile_pool(name="ps", bufs=2, space="PSUM") as pspool,
        ):
            # --- PE HAM warmup: dummy matmuls on memset scratch so the
            # clock-gate opens (1.2 -> 2.4 GHz) before the real stream.
            if N_WARMUP_MM:
                wa = wpool.tile([128, 128], F16, tag="warm_a")
                wb = wpool.tile([128, 512], F16, tag="warm_b")
                nc.gpsimd.memset(wa[:], 0.0)
                nc.gpsimd.memset(wb[:], 0.0)
                ps_warm = pspool.tile([128, 512], F32, tag="pw")
                for _ in range(N_WARMUP_MM):
                    nc.tensor.matmul(ps_warm[:], wa[:], wb[:],
                                     start=True, stop=True)

            # --- DMA in, all on the sync HWDGE queue; order matters:
            # xT first (it gates the whole ACT/DVE production chain),
            # btab (ACT biases), then C chunks smallest-first.
            xt = xpool.tile([128, FW], F32)
            nc.sync.dma_start(xt[:], xT[:])
            btab_sb = bpool.tile([128, 24], F32)
            nc.sync.dma_start(btab_sb[:], btab[:])
            chi = {}
            clo = {}
            kt0 = 0
            for ci, nkt in enumerate(CHUNK_KT):
                t = cpool.tile([128, nkt * 256], F16, tag=f"c{ci}")
                nc.sync.dma_start(
                    t[:], C2[:, kt0 * 256:(kt0 + nkt) * 256])
                for k in range(nkt):
                    chi[kt0 + k] = t[:, k * 128:(k + 1) * 128]
                    clo[kt0 + k] = t[:, (nkt + k) * 128:(nkt + k + 1) * 128]
                kt0 += nkt

            # ACT table preload: cheap Copy on the (early, tiny) btab tile.
            dummy = wpool.tile([128, 1], F32, tag="dummy")
            nc.scalar.copy(dummy[:], btab_sb[:, 0:1])

            # --- u = relu(scale*x + ubias), one op over both i-halves ---
            u2 = upool.tile([128, FW], F32)
            nc.scalar.activation(
                u2[:], xt[:], mybir.ActivationFunctionType.Relu,
                bias=btab_sb[:, 1:2], scale=float(scale),
            )

            # --- r_s = clamp(u - s, 0, 1) in fp16 ---
            r = []
            for s in range(S):
                rs = rpool.tile([128, FW], F16, tag="r")
                if s == 0:
                    # u >= 0, so clamp(u,0,1) = min(u,1)
                    nc.vector.tensor_scalar(
                        rs[:], u2[:], 1.0, None, mybir.AluOpType.min)
                elif s == S - 1:
                    # u < 20, so clamp(u-19,0,1) = relu(u-19)
                    nc.scalar.activation(
                        rs[:], u2[:], mybir.ActivationFunctionType.Relu,
                        bias=btab_sb[:, 1 + s:2 + s], scale=1.0)
                elif s in ACT_RELU_S:
                    # fp16 intermediate: values >= 1 still clamp to exactly
                    # 1.0 after quantization, t-entries keep fp16 precision,
                    # and the 16-bit input speeds up the DVE min.
                    w = wpool.tile([128, FW], F16, tag="w")
                    nc.scalar.activation(
                        w[:], u2[:], mybir.ActivationFunctionType.Relu,
                        bias=btab_sb[:, 1 + s:2 + s], scale=1.0)
                    nc.vector.tensor_scalar(
                        rs[:], w[:], 1.0, None, mybir.AluOpType.min)
                elif s <= N_GPS:
                    w = wpool.tile([128, FW], F16, tag="w")
                    nc.gpsimd.tensor_scalar(
                        w[:], u2[:], float(s), float(s),
                        mybir.AluOpType.max, mybir.AluOpType.subtract)
                    nc.gpsimd.tensor_scalar(
                        rs[:], w[:], 1.0, None, mybir.AluOpType.min)
                else:
                    w = wpool.tile([128, FW], F16, tag="w")
                    nc.vector.tensor_scalar(
                        w[:], u2[:], float(s), float(s),
                        mybir.AluOpType.max, mybir.AluOpType.subtract)
                    nc.vector.tensor_scalar(
                        rs[:], w[:], 1.0, None, mybir.AluOpType.min)
                r.append(rs)

            # --- matmuls: hi/lo interleaved per kt, two PSUM groups ---
            ps_hi = pspool.tile([O_LOC, B_LOC], F32, tag="ph")
            ps_lo = pspool.tile([O_LOC, B_LOC], F32, tag="pl")
            if os.environ.get("KAN_INTERLEAVE", "1") == "1":
                for kt in range(KT):
                    s, ih = kt // 2, kt % 2
                    rhs = r[s][:, ih * B_LOC:(ih + 1) * B_LOC]
                    nc.tensor.matmul(ps_hi[:], chi[kt], rhs,
                                     start=(kt == 0), stop=(kt == KT - 1))
                    nc.tensor.matmul(ps_lo[:], clo[kt], rhs,
                                     start=(kt == 0), stop=(kt == KT - 1))
            else:
                for ps, carr in ((ps_hi, chi), (ps_lo, clo)):
                    for kt in range(KT):
                        s, ih = kt // 2, kt % 2
                        rhs = r[s][:, ih * B_LOC:(ih + 1) * B_LOC]
                        nc.tensor.matmul(ps[:], carr[kt], rhs,
                                         start=(kt == 0), stop=(kt == KT - 1))

            # --- tail: out = ps_hi + ps_lo/2048 + bias ---
            t1 = opool.tile([O_LOC, B_LOC], F32, tag="t1")
            nc.vector.tensor_scalar(
                t1[:], ps_lo[:], 1.0 / LO_SCALE, btab_sb[:, 0:1],
                mybir.AluOpType.mult, mybir.AluOpType.add)
            out_sb = opool.tile([O_LOC, B_LOC], F32, tag="osb")
            nc.vector.tensor_tensor(
                out_sb[:], ps_hi[:], t1[:], mybir.AluOpType.add)
            nc.sync.dma_start(out[:], out_sb[:])
    nc.compile()
    return nc


_NC_CACHE: dict = {}


def _get_nc(scale: float, ubias: float) -> bass.Bass:
    key = (float(scale), float(ubias))
    if key not in _NC_CACHE:
        _NC_CACHE[key] = _build_nc(scale, ubias)
    return _NC_CACHE[key]


def prepare(x: np.ndarray, breakpoints: np.ndarray, values: np.ndarray):
    """Host prep: build the Bass graph (cached) + per-core input maps."""
    x = np.asarray(x, np.float32)
    breakpoints = np.asarray(breakpoints, np.float32)
    values = np.asarray(values, np.float32)

    # Grid affine params from the (shared) breakpoint row.
    bpr = breakpoints[0, 0].astype(np.float64)
    h = (bpr[-1] - bpr[0]) / S
    scale = float(1.0 / h)
    ubias = float(-bpr[0] / h)

    # Clamp-basis slopes, split into fp16 hi + scaled fp16 lo.
    Vf = values  # [O, I, S+1]
    M = (Vf[:, :, 1:] - Vf[:, :, :-1]).transpose(2, 0, 1)  # [S, O, I] f32
    M = np.ascontiguousarray(M, np.float32)
    Mhi = M.astype(np.float16)
    Mlo = ((M - Mhi.astype(np.float32)) * LO_SCALE).astype(np.float16)
    bias_o = Vf[:, :, 0].sum(axis=1, dtype=np.float64).astype(np.float32)

    # Per-core layouts.
    #   C*: [j, kt, o] fp16 with kt = 2*s + ih, j = i within half.
    #   xT: [j, ih*B_LOC + b] fp32.
    Mhi_r = Mhi.reshape(S, O_SPLIT, O_LOC, 2, 128)  # [s, oh, o, ih, j]
    Mlo_r = Mlo.reshape(S, O_SPLIT, O_LOC, 2, 128)
    xr = x.reshape(B_SPLIT, B_LOC, 2, 128)          # [bq, b, ih, j]

    in_maps = []
    for c in range(N_CORES):
        bq, oh = c % B_SPLIT, c // B_SPLIT
        # xr[bq] axes (b, ih, j) -> (j, ih, b) -> [128, FW]
        xT_c = np.ascontiguousarray(
            xr[bq].transpose(2, 1, 0)).reshape(128, FW)
        C_hi = np.ascontiguousarray(
            Mhi_r[:, oh].transpose(3, 0, 2, 1)).reshape(128, KT * 128)
        C_lo = np.ascontiguousarray(
            Mlo_r[:, oh].transpose(3, 0, 2, 1)).reshape(128, KT * 128)
        # Interleave hi/lo per DMA chunk: [hi kts of chunk][lo kts of chunk]
        blocks = []
        kt0 = 0
        for nkt in CHUNK_KT:
            blocks.append(C_hi[:, kt0 * 128:(kt0 + nkt) * 128])
            blocks.append(C_lo[:, kt0 * 128:(kt0 + nkt) * 128])
            kt0 += nkt
        C2_c = np.ascontiguousarray(np.concatenate(blocks, axis=1))
        bt = np.empty((128, 24), np.float32)
        bt[:, 0] = bias_o[oh * O_LOC:(oh + 1) * O_LOC]
        bt[:, 1] = ubias
        for k in range(1, 23):
            bt[:, 1 + k] = -float(k)
        in_maps.append({"xT": xT_c, "C2": C2_c, "btab": bt})

    nc = _get_nc(scale, ubias)
    return nc, in_maps


def kernel(x: np.ndarray, breakpoints: np.ndarray, values: np.ndarray,
           **_extra) -> np.ndarray:
    nc, in_maps = prepare(x, breakpoints, values)
    res = run_bass_kernel_spmd(nc, in_maps, list(range(N_CORES)))

    outf = np.empty((B, O), np.float32)
    for c in range(N_CORES):
        bq, oh = c % B_SPLIT, c // B_SPLIT
        outf[bq * B_LOC:(bq + 1) * B_LOC, oh * O_LOC:(oh + 1) * O_LOC] = \
            res.results[c]["out"].T
    return outf


if __name__ == "__main__":
    rng = np.random.default_rng(0)
    x = rng.uniform(-1, 1, (B, I)).astype(np.float32)
    bp = np.tile(np.linspace(-1, 1, S + 1, dtype=np.float32), (O, I, 1))
    v = (rng.standard_normal((O, I, S + 1)) * 0.1).astype(np.float32)
    out = kernel(x, bp, v)
    print("kernel ran, out:", out.shape, out.dtype, float(out.std()))
